# revision 11
# baseline (speedup 1.0000x reference)
"""Trainium2 Bass kernel for nn_ColbertAdapter (ColBERT late-interaction adapter).

Strategy (8 NeuronCores, single SPMD launch):
  - Context index (C=2048 entries) is sharded 256/core; queries replicated.
  - Per core: LN -> q/k/v projections (LN weights + 1/sqrt(dk) folded into the
    projection weights on host), scores computed transposed S^T[cu, t] so that
    exp(S) comes straight out of PSUM via ScalarE into the attnV-ready layout,
    MaxSim over U done on exp'd scores (exp is monotonic) as a bf16 TT-max
    tree on VectorE, attn@v via PE with a ones-column augmenting v so the
    softmax denominator falls out of the same matmul.
  - Softmax uses a fixed zero max-offset: LN output norm is exactly sqrt(D),
    so |logits| <= sqrt(D)^2 * smax(wq_eff) * smax(wk_eff) ~ 50 << 87, no
    overflow possible (checked on host via power iteration).
  - One ReduceScatter over a [8*520, 128] fp32 buffer merges both partial
    attention numerators and denominators and hands each core its own token
    shard; each core then runs divide -> wo -> LN4 -> wp for its 128 tokens.
"""

import os
import sys

try:
    import concourse  # noqa: F401
except ImportError:
    for p in ("/opt/trn_rl_repo", "/root/.axon_site/_ro/trn_rl_repo"):
        if os.path.isdir(p):
            sys.path.insert(0, p)
            break

import numpy as np
import ml_dtypes

import concourse.bass as bass
import concourse.mybir as mybir
from concourse import tile, bacc, bass_utils
from concourse.alu_op_type import AluOpType

BF16 = mybir.dt.bfloat16
F32 = mybir.dt.float32

NCORES = 8
B, T, C, U, D, P = 4, 256, 2048, 4, 512, 512
H = 8
DK = D // H
BT = B * T              # 1024 query tokens
CS = C // NCORES        # 256 contexts per core
CUS = CS * U            # 1024 key rows per core
TSH = BT // NCORES      # 128 tokens per core in the output shard
EPS = 1e-5

_CACHE = {}


def _emit_ln(nc, pools, x_tile, out_tile):
    """Plain layernorm over the free dim (512) of a [128, 512] fp32 tile,
    writing bf16. Uses bn_stats/bn_aggr for mean/var in ~1 pass."""
    small = pools["small"]
    stats6 = small.tile([128, 6], F32, tag="bns")
    nc.vector.bn_stats(stats6[:], x_tile[:])
    mv = small.tile([128, 2], F32, tag="bna")
    nc.vector.bn_aggr(mv[:], stats6[:])
    veps = small.tile([128, 1], F32, tag="veps")
    nc.vector.tensor_scalar_add(veps[:], mv[:, 1:2], EPS)
    std = small.tile([128, 1], F32, tag="std")
    nc.scalar.sqrt(std[:], veps[:])
    rstd = small.tile([128, 1], F32, tag="rstd")
    nc.vector.reciprocal(rstd[:], std[:])
    nc.vector.tensor_scalar(
        out_tile[:], x_tile[:], mv[:, 0:1], rstd[:],
        op0=AluOpType.subtract, op1=AluOpType.mult,
    )


def build_nc():
    nc = bacc.Bacc("TRN2", target_bir_lowering=False, debug=False,
                   num_devices=NCORES)

    # ---- DRAM I/O ----
    x_d = nc.dram_tensor("x", [BT, D], F32, kind="ExternalInput").ap()
    kin_d = nc.dram_tensor("kin", [CUS, D], F32, kind="ExternalInput").ap()
    vin_d = nc.dram_tensor("vin", [CS, D], F32, kind="ExternalInput").ap()
    w_d = {
        n: nc.dram_tensor(n, [D, D], BF16, kind="ExternalInput").ap()
        for n in ("wq", "wk", "wv", "wo", "wp")
    }
    bq_d = nc.dram_tensor("bq", [D], F32, kind="ExternalInput").ap()
    bk_d = nc.dram_tensor("bk", [D], F32, kind="ExternalInput").ap()
    bv_d = nc.dram_tensor("bv", [D], F32, kind="ExternalInput").ap()
    bo_d = nc.dram_tensor("bo", [D], BF16, kind="ExternalInput").ap()
    ind_d = nc.dram_tensor("ind", [8, 4 * TSH], F32, kind="ExternalInput").ap()
    eye_d = nc.dram_tensor("eye", [128, 128], BF16, kind="ExternalInput").ap()
    bp_d = nc.dram_tensor("bp", [D], BF16, kind="ExternalInput").ap()
    y_d = nc.dram_tensor("y", [TSH, P], F32, kind="ExternalOutput").ap()
    DEBUG = os.environ.get("KDEBUG", "0") == "1"
    if DEBUG:
        dbg_q = nc.dram_tensor("dbg_q", [128, 4 * BT], BF16,
                               kind="ExternalOutput").ap()
        dbg_k = nc.dram_tensor("dbg_k", [128, 4 * CUS], BF16,
                               kind="ExternalOutput").ap()
        dbg_bin = nc.dram_tensor("dbg_bin", [NCORES * 520, TSH], F32,
                                 kind="ExternalOutput").ap()
        dbg_bout = nc.dram_tensor("dbg_bout", [520, TSH], F32,
                                  kind="ExternalOutput").ap()
        dbg_xnt = nc.dram_tensor("dbg_xnt", [128, 4 * BT], BF16,
                                 kind="ExternalOutput").ap()
        dbg_ob = nc.dram_tensor("dbg_ob", [128, 4 * TSH], F32,
                                kind="ExternalOutput").ap()
        dbg_s = nc.dram_tensor("dbg_s", [8, TSH], F32,
                               kind="ExternalOutput").ap()
        dbg_rb = nc.dram_tensor("dbg_rb", [128, 4 * TSH], F32,
                                kind="ExternalOutput").ap()
        dbg_on = nc.dram_tensor("dbg_on", [128, 4 * TSH], BF16,
                                kind="ExternalOutput").ap()
        dbg_y1 = nc.dram_tensor("dbg_y1", [128, D], F32,
                                kind="ExternalOutput").ap()
        dbg_z = nc.dram_tensor("dbg_z", [128, D], BF16,
                               kind="ExternalOutput").ap()

    with tile.TileContext(nc) as tc:
        from contextlib import ExitStack
        ctx = ExitStack()
        with ctx:
            persist = ctx.enter_context(tc.tile_pool(name="persist", bufs=1))
            small = ctx.enter_context(tc.tile_pool(name="small", bufs=4))
            lnin = ctx.enter_context(tc.tile_pool(name="lnin", bufs=4))
            lnout = ctx.enter_context(tc.tile_pool(name="lnout", bufs=4))
            pall = ctx.enter_context(tc.tile_pool(name="pall", bufs=10))
            pmax = ctx.enter_context(tc.tile_pool(name="pmax", bufs=6))
            o65p = ctx.enter_context(tc.tile_pool(name="o65", bufs=3))
            psum = ctx.enter_context(
                tc.tile_pool(name="psum", bufs=2, space="PSUM"))
            dram = ctx.enter_context(
                tc.tile_pool(name="dram", bufs=1, space="DRAM"))
            pools = {"small": small}

            # ---- constants / weights into SBUF ----
            w_sb = {}
            for n in ("wq", "wk", "wv", "wo", "wp"):
                w_sb[n] = persist.tile([128, 4, D], BF16, tag=f"w_{n}",
                                       name=f"w_{n}")
                nc.sync.dma_start(
                    w_sb[n][:], w_d[n].rearrange("(b p) j -> p b j", p=128))
            bq_sb = persist.tile([128, 4], F32, tag="bq")
            nc.sync.dma_start(bq_sb[:], bq_d.rearrange("(b p) -> p b", p=128))
            bk_sb = persist.tile([128, 4], F32, tag="bk")
            nc.sync.dma_start(bk_sb[:], bk_d.rearrange("(b p) -> p b", p=128))
            bv_sb = persist.tile([128, 4], F32, tag="bv")
            nc.sync.dma_start(bv_sb[:], bv_d.rearrange("(b p) -> p b", p=128))
            bo_row = persist.tile([1, D], BF16, tag="bo_row")
            nc.sync.dma_start(bo_row[:], bo_d.rearrange("(o d) -> o d", o=1))
            bp_row = persist.tile([1, D], BF16, tag="bp_row")
            nc.sync.dma_start(bp_row[:], bp_d.rearrange("(o d) -> o d", o=1))
            ones_row = persist.tile([1, 128], BF16, tag="ones_row")
            nc.vector.memset(ones_row[:], 1.0)
            # head-indicator matrices (host-supplied constant) for
            # broadcasting the softmax denominator across each head's 64 o^T
            # rows via the PE: ind[h, b, p] = 1 iff head(b, p) == h
            ind = persist.tile([8, 4, 128], F32, tag="ind")
            nc.sync.dma_start(
                ind[:], ind_d.rearrange("h (b p) -> h b p", b=4))
            eye = persist.tile([128, 128], BF16, tag="eye")
            nc.sync.dma_start(eye[:], eye_d)

            tp_i = [0]

            def pe_transpose(dst_ap, src_ap):
                # [128,128] bf16 transpose on the PE; evac alternates DVE/ACT
                tp = psum.tile([128, 128], BF16, tag="attnv", name="tp")
                nc.tensor.transpose(tp[:], src_ap, eye[:])
                nc.vector.tensor_copy(dst_ap, tp[:])
                tp_i[0] += 1

            # ---- LN + transpose for x, k, v ----
            def ln_and_transpose(src_ap, n_tiles, dstT, tag):
                # dstT: [128, 4, n_tiles, 128] bf16 == srcLN^T blocks
                for i in range(n_tiles):
                    xt = lnin.tile([128, D], F32, tag="lnin")
                    nc.sync.dma_start(xt[:], src_ap[i * 128:(i + 1) * 128, :])
                    lt = lnout.tile([128, D], BF16, tag="lnout")
                    _emit_ln(nc, pools, xt, lt)
                    for jb in range(4):
                        pe_transpose(dstT[:, jb, i, :],
                                     lt[:, jb * 128:(jb + 1) * 128])

            xnT = persist.tile([128, 4, 8, 128], BF16, tag="xnT")
            ln_and_transpose(x_d, 8, xnT, "x")
            knT = persist.tile([128, 4, 8, 128], BF16, tag="knT")
            ln_and_transpose(kin_d, 8, knT, "k")
            vnT = persist.tile([128, 4, 2, 128], BF16, tag="vnT")
            ln_and_transpose(vin_d, 2, vnT, "v")

            # ---- projections ----
            # qT[j, t] = sum_d wq[d, j] * xn^T[d, t]   (jt-tiles x t-chunks)
            qT = persist.tile([128, 4, BT], BF16, tag="qT")
            kT = persist.tile([128, 4, CUS], BF16, tag="kT")
            for (dstT, wname, srcT, bias, nchunk) in (
                    (qT, "wq", xnT, bq_sb, 2), (kT, "wk", knT, bk_sb, 2)):
                for jt in range(4):
                    for tch in range(nchunk):
                        ps = psum.tile([128, 1024], F32, tag="wide")
                        for dt in range(4):
                            nc.tensor.matmul(
                                ps[:, :512],
                                lhsT=w_sb[wname][:, dt,
                                                 jt * 128:(jt + 1) * 128],
                                rhs=srcT[:, dt, tch * 4:(tch + 1) * 4, :],
                                start=(dt == 0), stop=(dt == 3))
                        nc.vector.tensor_scalar_add(
                            dstT[:, jt, tch * 512:(tch + 1) * 512],
                            ps[:, :512], bias[:, jt:jt + 1])

            # v[c, hd] with ones column per head -> v_sb[ct]: [128, 8, 65]
            v_sb = []
            for ct in range(2):
                vt = persist.tile([128, 8, 65], BF16, tag=f"v_sb{ct}")
                ps = psum.tile([128, 1024], F32, tag="wide")
                for dt in range(4):
                    nc.tensor.matmul(
                        ps[:, :512],
                        lhsT=vnT[:, dt, ct, :],
                        rhs=w_sb["wv"][:, dt, :],
                        start=(dt == 0), stop=(dt == 3))
                nc.vector.tensor_copy(
                    vt[:, :, 0:64],
                    ps[:, :512].rearrange("p (h e) -> p h e", h=8))
                nc.vector.memset(vt[:, :, 64:65], 1.0)
                v_sb.append(vt)

            # ---- per-head: scores^T -> exp -> U-max -> attn@v_aug ----
            bounce_ins = [
                dram.tile([NCORES, 130, TSH], F32, name=f"bin{i}")
                for i in range(4)
            ]
            bounce_outs = [
                dram.tile([130, TSH], F32, name=f"bout{i}")
                for i in range(4)
            ]
            for h in range(H):
                hp = (h % 2) * 64
                jt = h // 2
                p_all = []
                for r in range(8):  # cu-tile: rows r*128+p ; u = r//2, c2 = r%2
                    ps = psum.tile([128, 1024], F32, tag="wide")
                    for tch in range(2):
                        nc.tensor.matmul(
                            ps[:, tch * 512:(tch + 1) * 512],
                            lhsT=kT[hp:hp + 64, jt, r * 128:(r + 1) * 128],
                            rhs=qT[hp:hp + 64, jt, tch * 512:(tch + 1) * 512],
                            start=True, stop=True)
                    pt = pall.tile([128, 1024], BF16, tag="pall")
                    nc.scalar.activation(
                        pt[:], ps[:], mybir.ActivationFunctionType.Exp)
                    p_all.append(pt)
                pm = []
                for c2 in range(2):
                    t1 = pmax.tile([128, 1024], BF16, tag="pm")
                    nc.vector.tensor_max(t1[:], p_all[c2][:], p_all[2 + c2][:])
                    t2 = pmax.tile([128, 1024], BF16, tag="pm")
                    nc.vector.tensor_max(t2[:], p_all[4 + c2][:],
                                         p_all[6 + c2][:])
                    t3 = pmax.tile([128, 1024], BF16, tag="pm")
                    nc.vector.tensor_max(t3[:], t1[:], t2[:])
                    pm.append(t3)
                pso = psum.tile([65, 1024], F32, tag="attnv")
                for tch in range(2):
                    for c2 in range(2):
                        nc.tensor.matmul(
                            pso[:, tch * 512:(tch + 1) * 512],
                            lhsT=v_sb[c2][:, h, :],
                            rhs=pm[c2][:, tch * 512:(tch + 1) * 512],
                            start=(c2 == 0), stop=(c2 == 1))
                o65 = o65p.tile([65, 1024], F32, tag="o65")
                nc.vector.tensor_copy(o65[:], pso[:])
                b_in = bounce_ins[h // 2]
                hh = h % 2
                nc.sync.dma_start(
                    b_in[:, hh * 65:(hh + 1) * 65, :].rearrange(
                        "s r t -> r s t"),
                    o65.rearrange("r (s t) -> r s t", s=NCORES))
                if h % 2 == 1:
                    nc.gpsimd.collective_compute(
                        "ReduceScatter", AluOpType.add,
                        replica_groups=[list(range(NCORES))],
                        ins=[bounce_ins[h // 2].rearrange("s r t -> (s r) t")],
                        outs=[bounce_outs[h // 2].opt()],
                    )

            if DEBUG:
                nc.sync.dma_start(
                    dbg_q.rearrange("p (b t) -> p b t", b=4), qT[:])
                nc.sync.dma_start(
                    dbg_k.rearrange("p (b t) -> p b t", b=4), kT[:])
                nc.sync.dma_start(
                    dbg_xnt.rearrange("p (b i t) -> p b i t", b=4, i=8),
                    xnT[:])
                nc.sync.dma_start(
                    dbg_bin[0:NCORES * 260, :].rearrange(
                        "(s r) t -> s r t", s=NCORES), bounce_in_a[:])
                nc.sync.dma_start(
                    dbg_bin[NCORES * 260:, :].rearrange(
                        "(s r) t -> s r t", s=NCORES), bounce_in_b[:])


            # ---- readback merged o^T (+denominators) for our token shard ----
            bviews = [bo_.rearrange("(h j) t -> h j t", j=65)
                      for bo_ in bounce_outs]
            ob = persist.tile([128, 4, TSH], F32, tag="ob")
            s_sb = persist.tile([8, TSH], F32, tag="s_sb")
            for h in range(H):
                eng = nc.sync if h % 2 == 0 else nc.scalar
                eng.dma_start(
                    ob[(h % 2) * 64:(h % 2) * 64 + 64, h // 2, :],
                    bviews[h // 2][h % 2, 0:64, :])
            for i in range(4):
                nc.sync.dma_start(s_sb[2 * i:2 * i + 2, :],
                                  bviews[i][:, 64, :])

            # broadcast denominators to each head's 64 rows via the PE,
            # then take the reciprocal
            ps_s = psum.tile([128, 1024], F32, tag="wide")
            for bb in range(4):
                nc.tensor.matmul(ps_s[:, bb * TSH:(bb + 1) * TSH],
                                 lhsT=ind[:, bb, :], rhs=s_sb[:],
                                 start=True, stop=True)
            rb = persist.tile([128, 4, TSH], F32, tag="rb")
            nc.vector.reciprocal(
                rb.rearrange("p b t -> p (b t)"), ps_s[:, :4 * TSH])
            o_n = persist.tile([128, 4, TSH], BF16, tag="o_n")
            for b in range(4):
                t = small.tile([128, TSH], F32, tag="odiv")
                nc.vector.tensor_mul(t[:], ob[:, b, :], rb[:, b, :])
                nc.vector.tensor_scalar_add(
                    o_n[:, b, :], t[:], bv_sb[:, b:b + 1])

            if DEBUG:
                nc.sync.dma_start(
                    dbg_ob.rearrange("p (b t) -> p b t", b=4), ob[:])
                nc.sync.dma_start(dbg_s.opt(), s_sb[:])
                nc.sync.dma_start(
                    dbg_rb.rearrange("p (b t) -> p b t", b=4), rb[:])
                nc.sync.dma_start(
                    dbg_on.rearrange("p (b t) -> p b t", b=4), o_n[:])
            # wo projection + bo
            psy = psum.tile([128, 1024], F32, tag="wide")
            for b in range(4):
                nc.tensor.matmul(psy[:, :512], lhsT=o_n[:, b, :],
                                 rhs=w_sb["wo"][:, b, :],
                                 start=(b == 0), stop=False)
            nc.tensor.matmul(psy[:, :512], lhsT=ones_row[:],
                             rhs=bo_row[:], start=False, stop=True)
            y1 = persist.tile([128, D], F32, tag="y1")
            nc.vector.tensor_copy(y1[:], psy[:, :512])

            if DEBUG:
                nc.sync.dma_start(dbg_y1.opt(), y1[:])
            # LN4 -> z (bf16), transpose, wp projection + bp
            z = persist.tile([128, D], BF16, tag="z")
            _emit_ln(nc, pools, y1, z)
            zT = persist.tile([128, 4, TSH], BF16, tag="zT")
            for b in range(4):
                pe_transpose(zT[:, b, :], z[:, b * 128:(b + 1) * 128])
            if DEBUG:
                nc.sync.dma_start(dbg_z.opt(), z[:])
            psy2 = psum.tile([128, 1024], F32, tag="wide")
            for b in range(4):
                nc.tensor.matmul(psy2[:, :512], lhsT=zT[:, b, :],
                                 rhs=w_sb["wp"][:, b, :],
                                 start=(b == 0), stop=False)
            nc.tensor.matmul(psy2[:, :512], lhsT=ones_row[:],
                             rhs=bp_row[:], start=False, stop=True)
            yt = persist.tile([128, P], F32, tag="yt")
            nc.vector.tensor_copy(yt[:], psy2[:, :512])
            nc.sync.dma_start(y_d[:], yt[:])

    nc.compile()
    return nc


def _make_ind():
    ind = np.zeros((8, 4, TSH), np.float32)
    for h in range(8):
        ind[h, h // 2, (h % 2) * 64:(h % 2) * 64 + 64] = 1.0
    return ind.reshape(8, 4 * TSH)


def _prep_host(inputs):
    """Fold LN weights/biases and 1/sqrt(dk) into projection weights; build
    per-core input maps."""
    f32 = np.float32
    bf16 = ml_dtypes.bfloat16
    me = np.ascontiguousarray(inputs["model_embed"], dtype=f32).reshape(BT, D)
    kin = np.asarray(inputs["context_embed_key"], dtype=f32)
    vin = np.asarray(inputs["context_embed_value"], dtype=f32)
    g = lambda n: np.asarray(inputs[n], dtype=f32)

    scale = 1.0 / np.sqrt(DK)
    wq_eff = (g("ln1_w")[:, None] * g("wq")) * scale
    bq_eff = (g("ln1_b") @ g("wq") + g("bq")) * scale
    wk_eff = g("ln2_w")[:, None] * g("wk")
    bk_eff = g("ln2_b") @ g("wk") + g("bk")
    wv_eff = g("ln3_w")[:, None] * g("wv")
    bv_eff = g("ln3_b") @ g("wv") + g("bv")
    wo_eff = g("wo")
    bo_eff = g("bo")
    wp_eff = g("ln4_w")[:, None] * g("wp")
    bp_eff = g("ln4_b") @ g("wp") + g("bp")

    # overflow guard for the zero-offset softmax: |logits| must stay << 87
    def smax(w):
        v = np.random.RandomState(0).randn(w.shape[1]).astype(f32)
        for _ in range(20):
            v = w.T @ (w @ v)
            v /= np.linalg.norm(v)
        return np.linalg.norm(w @ v)
    bound = ((np.sqrt(D) * smax(wq_eff) + np.linalg.norm(bq_eff))
             * (np.sqrt(D) * smax(wk_eff) + np.linalg.norm(bk_eff)))
    assert bound < 80.0, f"logit bound {bound} too large for exp without max"

    common = {
        "x": me,
        "wq": wq_eff.astype(bf16), "wk": wk_eff.astype(bf16),
        "wv": wv_eff.astype(bf16), "wo": wo_eff.astype(bf16),
        "wp": wp_eff.astype(bf16),
        "bq": bq_eff, "bk": bk_eff, "bv": bv_eff,
        "bo": bo_eff.astype(bf16), "bp": bp_eff.astype(bf16),
        "ind": _make_ind(),
        "eye": np.eye(128, dtype=bf16),
    }
    in_maps = []
    for c in range(NCORES):
        ksh = kin[c * CS:(c + 1) * CS]             # [CS, U, D]
        ksh = np.ascontiguousarray(
            ksh.transpose(1, 0, 2).reshape(CUS, D))  # u-major rows
        vsh = np.ascontiguousarray(vin[c * CS:(c + 1) * CS])
        m = dict(common)
        m["kin"] = ksh
        m["vin"] = vsh
        in_maps.append(m)
    return in_maps


def kernel(**inputs) -> np.ndarray:
    if "nc" not in _CACHE:
        _CACHE["nc"] = build_nc()
    nc = _CACHE["nc"]
    in_maps = _prep_host(inputs)
    res = bass_utils.run_bass_kernel_spmd(
        nc, in_maps, core_ids=list(range(NCORES)))
    y = np.concatenate([res.results[c]["y"] for c in range(NCORES)], axis=0)
    return y.reshape(B, T, P).astype(np.float32)


if __name__ == "__main__":
    # quick smoke: random inputs of the right shapes
    rng = np.random.RandomState(0)
    print("building...")
    build_nc()
    print("ok")
